# revision 44
# baseline (speedup 1.0000x reference)
"""Trainium2 Bass kernel for nn_CapsuleUnit (capsule routing) — x-basis.

Reference math (full problem, 10 routing iterations):
    u = einsum('bic,co->bio', x, W) + bias          # [b, in_caps, out]
    repeat 10x:
        cij = softmax(c, axis=in_caps)              # shared across batch
        sj  = sum_i u * cij                         # [b, out]
        vj  = sj * n / (1 + n^2),  n = ||sj||       # squash
        c  += einsum('bio,bo->i', u, vj)            # agreement over batch+out
    return vj (last iteration)

Key restructure vs the u-basis baseline: u is NEVER materialized. Using
u = x@W + bias:
    pass 1:  sj = (sum_i e_i x[b,i,:]) @ W + S*bias      (e = exp(c-max), S=sum e)
    pass 2:  upd_i = sum_b x[b,i,:] . (W @ vj[b])        (bias term is a constant
             shift over i -> softmax-invariant -> dropped)
This removes the 61us setup matmul (u = x@W) entirely. Each pass is 288
1-column PE matmuls against SBUF-resident x in the appropriate layout:
    xi  [p=i%128, t, b, c]  (i on partitions)  for pass 1   (fp8 + bf16 copy)
    xt8 [p=c%128, ct, b, i] (c on partitions)  for pass 2   (fp8)
Routing iterations 1-8 use fp8 x (routing is precision-insensitive: softmax
saturates); the final iteration's pass 1 refines fp8 x with a second fp8
residual tensor xR8 = fp8(16*(x - fp8(x))), accumulated into the same PSUM
group against e/16 -- better than bf16 precision at half the DMA bytes.
Verified vs reference on device: rel err ~2.7e-3.

The per-iteration cross-batch sum of upd (4.6KB) uses direct remote-DMA
broadcasts (XOR-relative single-dest sends into per-sender gather slots)
+ local reduce instead of a collective_compute AllGather: ~1.5us vs ~16us.
Parity-double-buffered gather tiles; safety argument: my trigger for round
k requires my reduce of round k-1, which required every peer's round-(k-1)
send, which required their round-(k-2) reduce -- i.e. every peer has
finished READING gath[k%2] before anyone can overwrite it.
"""
import os
import sys
import numpy as np

sys.path.insert(0, "/opt/trn_rl_repo")

import ml_dtypes  # noqa: E402

import concourse.bass as bass  # noqa: E402
import concourse.bass_isa as bass_isa  # noqa: E402
import concourse.bacc as bacc  # noqa: E402
import concourse.mybir as mybir  # noqa: E402
import concourse.tile as tile  # noqa: E402
from concourse.bass_utils import run_bass_kernel_spmd  # noqa: E402

P = 128
F32 = mybir.dt.float32
BF16 = mybir.dt.bfloat16
F8 = mybir.dt.float8e4
AX = mybir.AxisListType
ALU = mybir.AluOpType
ACTF = mybir.ActivationFunctionType

# full problem config
FULL = dict(n_cores=8, B=8, IC=1152, CH=512, OC=512, iters=10)


def build_nc(n_cores, B, IC, CH, OC, iters):
    """Build the per-core SPMD program. All cores run identical code."""
    T = IC // P       # in_caps tiles
    CT = CH // P      # in_ch tiles
    OT = OC // P      # out_ch tiles

    nc = bacc.Bacc("TRN2", target_bir_lowering=False, debug=False,
                   num_devices=n_cores)
    # remote-DMA sem waits are hand-managed; the rust race detector's
    # valid-waits bookkeeping for single-dest broadcasts is inconsistent
    # with the executor's delivery counts (spurious SemaphoreRace).
    nc.detect_race_conditions = False

    xt8_d = nc.dram_tensor("xT8", [CH, B * IC], F8, kind="ExternalInput")
    xi8_d = nc.dram_tensor("xI8", [IC, B * CH], F8, kind="ExternalInput")
    xr8_d = nc.dram_tensor("xR8", [IC, B * CH], F8, kind="ExternalInput")
    w_d = nc.dram_tensor("Wt", [CH, OC], BF16, kind="ExternalInput")
    wt8_d = nc.dram_tensor("WT8", [OC, CH], F8, kind="ExternalInput")
    bias_d = nc.dram_tensor("bias", [OC], F32, kind="ExternalInput")
    coef_d = nc.dram_tensor("coeffs", [IC], F32, kind="ExternalInput")
    xbar_d = nc.dram_tensor("xbar", [CH, B], BF16, kind="ExternalInput")
    out_d = nc.dram_tensor("vj_out", [B, OC], F32, kind="ExternalOutput")
    ident_d = nc.inline_tensor(np.eye(P, dtype=np.float32), name="ident128")

    rsem = nc.alloc_semaphore("rsem")
    lsem = nc.alloc_semaphore("lsem")
    patches = []

    def defer_wait(bi, sem, thr):
        # Encoded as >=0 so the single-core tile scheduling pass (which never
        # sees remote increments) can't deadlock; patched to the real
        # threshold after scheduling, before compile.
        bi.wait_op(sem, 0, "sem-ge")
        patches.append((bi.ins.sync_info.on_wait[-1], thr))
        return bi

    with tile.TileContext(nc) as tc:
        with tc.tile_pool(name="big", bufs=1) as big, \
             tc.tile_pool(name="cst", bufs=1) as cst, \
             tc.tile_pool(name="sm", bufs=2) as sm, \
             tc.tile_pool(name="ps_loop", bufs=1, space="PSUM") as psl, \
             tc.tile_pool(name="ps_sm", bufs=3, space="PSUM") as pss:

            # ---- persistent SBUF state ----
            w_sb = cst.tile([P, CT, OC], BF16)
            wt8_sb = cst.tile([P, OT, CH], F8)
            bias_sb = cst.tile([P, OT], F32)
            bias_row = cst.tile([1, OC], F32)
            ident = cst.tile([P, P], F32)
            xbar_sb = cst.tile([P, CT, B], BF16)
            ones_col = cst.tile([P, 1], F32)
            ones_f8 = cst.tile([P, 1], F8)
            ones_bf = cst.tile([P, 1], BF16)
            ones_rp = cst.tile([1, P], F32)
            ones_rn = cst.tile([1, P], F32)
            c_buf = [cst.tile([P, T], F32, tag="c0", name="c0"),
                     cst.tile([P, T], F32, tag="c1", name="c1")]
            gath = [cst.tile([P, 9, T], F32, tag="g0", name="g0"),
                    cst.tile([P, 9, T], F32, tag="g1", name="g1")]
            xt8 = big.tile([P, CT, B, IC], F8)
            HB = B // 2
            xi8h = [big.tile([P, T, HB, CH], F8, tag="xi8a", name="xi8a"),
                    big.tile([P, T, HB, CH], F8, tag="xi8b", name="xi8b")]
            xr8 = big.tile([P, T, B, CH], F8)

            # consts first (cheap; bias comes in column form and is
            # transposed on-chip to avoid a 1.6us single-partition DMA),
            # then the big x loads in consumption order: xt8 gates it0's
            # pass 2, xi8 (t-chunked so pass 1 streams behind it) gates
            # it1's pass 1, xib only it9.
            nc.sync.dma_start(out=w_sb[:], in_=w_d[:].rearrange(
                "(ct p) o -> p ct o", p=P))
            nc.sync.dma_start(out=xbar_sb[:], in_=xbar_d[:].rearrange(
                "(ct p) b -> p ct b", p=P))
            nc.sync.dma_start(out=bias_sb[:], in_=bias_d[:].rearrange(
                "(ot p) -> p ot", p=P))
            nc.sync.dma_start(out=c_buf[0][:], in_=coef_d[:].rearrange(
                "(t p) -> p t", p=P))
            nc.sync.dma_start(out=ident[:], in_=ident_d[:])
            nc.sync.dma_start(out=wt8_sb[:], in_=wt8_d[:].rearrange(
                "(ot p) c -> p ot c", p=P))
            nc.vector.memset(ones_col[:], 1.0)
            nc.vector.memset(ones_f8[:], 1.0)
            nc.vector.memset(ones_bf[:], 1.0)
            nc.vector.memset(ones_rp[:], 1.0)
            nc.vector.memset(ones_rn[:], -1.0)
            nc.sync.dma_start(out=xt8[:], in_=xt8_d[:].rearrange(
                "(ct p) (b i) -> p ct b i", p=P, b=B))
            xi8_view = xi8_d[:].rearrange("(t p) (b c) -> p t b c", p=P, b=B)
            nc.sync.dma_start(out=xi8h[0][:], in_=xi8_view[:, :, 0:HB, :])
            nc.sync.dma_start(out=xi8h[1][:], in_=xi8_view[:, :, HB:B, :])
            nc.sync.dma_start(out=xr8[:], in_=xr8_d[:].rearrange(
                "(t p) (b c) -> p t b c", p=P, b=B))

            # bias_row[1, OC] = bias_sb^T via 4 PE transposes + ACT copies
            for ot in range(OT):
                brp = pss.tile([1, P], F32, tag="psml")
                nc.tensor.transpose(brp[:], bias_sb[:, ot:ot + 1], ident[:])
                nc.scalar.copy(bias_row[:, ot * P:ot * P + P], brp[:])

            # ---- persistent PSUM ----
            s_xp = psl.tile([P, CT * B], F32, tag="s_xp")
            sjT = psl.tile([P, OT * B], F32, tag="sjT")
            wvp = psl.tile([P, CT * B], F32, tag="wvp")
            upd2 = psl.tile([P, T * B], F32, tag="upd2")
            galp = psl.tile([P, T * B], F32, tag="galp")

            for it in range(iters):
                c_cur = c_buf[it % 2]
                last = (it == iters - 1)
                first = (it == 0)

                if first:
                    # c0 constant => softmax uniform: sj = xbar@W + bias via
                    # the host-prereduced xbar = mean_i(x).
                    for ot in range(OT):
                        col = sjT[:, B * ot:B * ot + B]
                        for ct in range(CT):
                            nc.tensor.matmul(
                                col, w_sb[:, ct, ot * P:ot * P + P],
                                xbar_sb[:, ct, :],
                                start=(ct == 0), stop=False)
                        nc.tensor.matmul(
                            col, bias_row[:, ot * P:ot * P + P],
                            ones_rp[:, 0:B], start=False, stop=True)
                else:
                    # global max of c (softmax stability; c grows unbounded):
                    # per-partition max on DVE, cross-partition max + bcast on
                    # the otherwise-idle Pool engine, negate on ACT (in-order
                    # with the exp, no extra sem hop)
                    cmax = sm.tile([P, 1], F32, tag="cmax")
                    nc.vector.reduce_max(cmax[:], c_cur[:], axis=AX.X)
                    mall = sm.tile([P, 1], F32, tag="mall")
                    nc.gpsimd.partition_all_reduce(
                        mall[:], cmax[:], P, bass_isa.ReduceOp.max)
                    ngm = sm.tile([P, 1], F32, tag="ngm")
                    nc.gpsimd.tensor_scalar_mul(ngm[:], mall[:], -1.0)
                    # e = exp(c - max) unnormalized; 1/S folds into squash
                    e_q = sm.tile([P, T], BF16 if last else F8, tag="e_q")
                    nc.scalar.activation(e_q[:], c_cur[:], ACTF.Exp,
                                         bias=ngm[:], scale=1.0)
                    # S = sum(e) via PE (runs beside pass 1a): per-t partition
                    # sums, reduce, broadcast to 8 cols + 1/S as [1,1] and
                    # replicated [P,1] (ACT scale operand), S^2 for squash
                    spt = pss.tile([1, T], F32, tag="psml")
                    nc.tensor.matmul(spt[:], (ones_bf if last else ones_f8)[:],
                                     e_q[:], start=True, stop=True)
                    s1 = sm.tile([1, 1], F32, tag="s1")
                    nc.vector.tensor_reduce(s1[:], spt[:], axis=AX.X,
                                            op=ALU.add)
                    ssp8 = pss.tile([1, B], F32, tag="psml")
                    nc.tensor.matmul(ssp8[:], s1[:], ones_rp[:, 0:B],
                                     start=True, stop=True)
                    s8_sb = sm.tile([1, B], F32, tag="s8_sb")
                    nc.vector.tensor_copy(s8_sb[:], ssp8[:])
                    rtot = sm.tile([1, 1], F32, tag="rtot")
                    nc.vector.reciprocal(rtot[:], s1[:])
                    s2t = sm.tile([1, 1], F32, tag="s2t")
                    nc.vector.tensor_tensor(s2t[:], s1[:], s1[:], op=ALU.mult)
                    if not last:
                        rtp = pss.tile([P, 1], F32, tag="psml")
                        nc.tensor.matmul(rtp[:], ones_rp[:], rtot[:],
                                         start=True, stop=True)
                        rts = sm.tile([P, 1], F32, tag="rts")
                        nc.scalar.copy(rts[:], rtp[:])

                    # pass 1a: s_x[c, (ct,b)] = sum_i x[b,i,c] e_i
                    # pass 1b: sjT[o, (ot,b)] = s_x @ W + S*bias
                    s_x_bf = sm.tile([P, CT * B], BF16, tag="s_x_bf")
                    if last:
                        # fp8 residual refinement: the xr8 matmuls against
                        # e/16 continue the same PSUM accumulation group
                        e16 = sm.tile([P, T], BF16, tag="e16")
                        nc.vector.tensor_scalar(e16[:], e_q[:], 0.0625, None,
                                                op0=ALU.mult)
                    for b in range(B):
                        xsl = xi8h[b // HB][:, :, b % HB, :]
                        for ct in range(CT):
                            col = s_xp[:, ct * B + b:ct * B + b + 1]
                            for t in range(T):
                                nc.tensor.matmul(
                                    col, xsl[:, t, ct * P:ct * P + P],
                                    e_q[:, t:t + 1],
                                    start=(t == 0),
                                    stop=(not last and t == T - 1))
                            if last:
                                for t in range(T):
                                    nc.tensor.matmul(
                                        col, xr8[:, t, b, ct * P:ct * P + P],
                                        e16[:, t:t + 1],
                                        start=False, stop=(t == T - 1))
                    nc.vector.tensor_copy(s_x_bf[:], s_xp[:])
                    for ot in range(OT):
                        col = sjT[:, B * ot:B * ot + B]
                        for ct in range(CT):
                            nc.tensor.matmul(
                                col, w_sb[:, ct, ot * P:ot * P + P],
                                s_x_bf[:, ct * B:ct * B + B],
                                start=(ct == 0), stop=False)
                        nc.tensor.matmul(
                            col, bias_row[:, ot * P:ot * P + P],
                            s8_sb[:], start=False, stop=True)

                # squash scalars on RAW y = ||sjT||^2 = S^2 ||sj||^2:
                #   z ~ rsqrt(Y) (DVE Newton; no ACT table thrash)
                #   g_true/S = 1/(z * (S^2 + Y))     [it0: S=1]
                # sq -> PE colsum -> one DVE grouped reduce; meanwhile ACT
                # emits sj8 = fp8(sjT/S) so the W^T matmuls (Wv before the
                # g scaling) run off the critical DVE chain.
                sq = sm.tile([P, OT * B], F32 if last else BF16, tag="sq")
                nc.scalar.activation(sq[:], sjT[:], ACTF.Square)
                ysump = pss.tile([1, OT * B], F32, tag="psml")
                nc.tensor.matmul(ysump[:], (ones_col if last else ones_bf)[:],
                                 sq[:], start=True, stop=True)
                y_sb = sm.tile([1, B], F32, tag="y_sb")
                nc.vector.tensor_reduce(
                    y_sb[:], ysump[:].rearrange("one (ot b) -> one b ot",
                                                ot=OT),
                    axis=AX.X, op=ALU.add)
                if not last:
                    sj8 = sm.tile([P, OT * B], F8, tag="sj8")
                    if first:
                        nc.scalar.activation(sj8[:], sjT[:], ACTF.Identity)
                    else:
                        nc.scalar.activation(sj8[:], sjT[:], ACTF.Identity,
                                             scale=rts[:])
                    # Wv_raw[c, (ct,b)] = sum_o W[c,o] sj8[o,(ot,b)]
                    for ct in range(CT):
                        for ot in range(OT):
                            nc.tensor.matmul(
                                wvp[:, ct * B:ct * B + B],
                                wt8_sb[:, ot, ct * P:ct * P + P],
                                sj8[:, ot * B:ot * B + B],
                                start=(ot == 0), stop=(ot == OT - 1))
                zb = sm.tile([1, B], F32, tag="zb")
                nc.vector.tensor_scalar(
                    zb[:].bitcast(mybir.dt.int32),
                    y_sb[:].bitcast(mybir.dt.int32),
                    -0.5, 1597463007.0, op0=ALU.mult, op1=ALU.add)
                zt = sm.tile([1, B], F32, tag="zt")
                for _nr in range(2 if last else 1):
                    nc.vector.tensor_tensor(zt[:], zb[:], zb[:], op=ALU.mult)
                    nc.vector.tensor_tensor(zt[:], zt[:], y_sb[:], op=ALU.mult)
                    nc.vector.tensor_scalar(zt[:], zt[:], -0.5, 1.5,
                                            op0=ALU.mult, op1=ALU.add)
                    nc.vector.tensor_tensor(zb[:], zb[:], zt[:], op=ALU.mult)
                dz = sm.tile([1, B], F32, tag="dz")
                if first:
                    nc.vector.scalar_tensor_tensor(
                        dz[:], y_sb[:], 1.0, zb[:], op0=ALU.add, op1=ALU.mult)
                else:
                    nc.vector.scalar_tensor_tensor(
                        dz[:], y_sb[:], s2t[:], zb[:],
                        op0=ALU.add, op1=ALU.mult)
                gi = sm.tile([1, B], F32, tag="gi")
                nc.vector.reciprocal(gi[:], dz[:])
                if last:
                    # output needs vj = sjT * (g/S): fold 1/S into g here
                    g2 = sm.tile([1, B], F32, tag="g2")
                    nc.vector.tensor_scalar(g2[:], gi[:], rtot[:], None,
                                            op0=ALU.mult)
                    alps = galp[:, 0:OT * B]
                    for ot in range(OT):
                        nc.tensor.matmul(alps[:, B * ot:B * ot + B],
                                         ones_rp[:], g2[:],
                                         start=True, stop=True)
                    al_sb = sm.tile([P, OT * B], F32, tag="al_sb")
                    nc.vector.tensor_copy(al_sb[:], alps)
                    vjf = sm.tile([P, B, OT], F32, tag="vjf")
                    nc.vector.tensor_tensor(
                        vjf[:].rearrange("p b ot -> p ot b"),
                        sjT[:].rearrange("p (ot b) -> p ot b", b=B),
                        al_sb[:].rearrange("p (ot b) -> p ot b", b=B),
                        op=ALU.mult)
                    nc.sync.dma_start(
                        out=out_d[:].rearrange("b (ot p) -> p b ot", p=P),
                        in_=vjf[:])
                    break
                # pass 2 on UNSCALED Wv (runs under the squash-scalar chain):
                # upd2[i%128, (t,b)] = sum_c x[b,i,c] Wv_raw[b,c]
                wv8 = sm.tile([P, CT * B], F8, tag="wv8")
                nc.scalar.copy(wv8[:], wvp[:])
                for t in range(T):
                    for b in range(B):
                        col = upd2[:, t * B + b:t * B + b + 1]
                        for ct in range(CT):
                            nc.tensor.matmul(
                                col, xt8[:, ct, b, t * P:t * P + P],
                                wv8[:, ct * B + b:ct * B + b + 1],
                                start=(ct == 0), stop=(ct == CT - 1))
                # g-weighted sum over b straight into the gather slot
                for t in range(T):
                    nc.tensor.matmul(galp[:, t * B:t * B + B], ones_rp[:],
                                     gi[:], start=True, stop=True)
                gl_sb = sm.tile([P, T * B], F32, tag="gl_sb")
                nc.vector.tensor_copy(gl_sb[:], galp[:])
                u2w = sm.tile([P, T * B], F32, tag="u2w")
                nc.vector.tensor_tensor(u2w[:], upd2[:], gl_sb[:], op=ALU.mult)

                # ---- cross-core allgather of upd via remote DMA ----
                g = gath[it % 2]
                cp = nc.vector.tensor_reduce(
                    g[:, 0, :], u2w[:].rearrange("p (t b) -> p t b", t=T),
                    axis=AX.X, op=ALU.add)
                if it >= 2:
                    # my sends of round it-2 (which read this parity buffer's
                    # slot 0) must have left before overwriting
                    defer_wait(cp, lsem, 112 * (it - 1))
                nc.vector.tensor_copy(g[:, 8, :], c_cur[:])
                for d in range(1, 8):
                    rds = [None] * 8
                    rds[d] = (0, d)
                    nc.gpsimd.remote_dma_broadcast(
                        out_ap=g[:, d, :], in_ap=g[:, 0, :],
                        remote_sem=rsem, local_sem=lsem, rdests=rds)
                nc.gpsimd.trigger_dma(count=None,
                                      signals_writable=[g[:, 1:8, :]])
                red = nc.vector.tensor_reduce(
                    c_buf[(it + 1) % 2][:],
                    g[:].rearrange("p r t -> p t r"),
                    axis=AX.X, op=ALU.add)
                defer_wait(red, rsem, 14 * (it + 1))

    for sw, thr in patches:
        sw.wait_value = thr
    nc.compile()
    # the deferred waits must survive lowering: verify they are encoded
    n_found = 0
    for fn in nc.m.functions:
        for bb in fn.blocks:
            for ins in bb.instructions:
                if ins.sync_info:
                    for w in ins.sync_info.on_wait:
                        if w.ant_name in ("rsem", "lsem") and w.wait_value > 0:
                            n_found += 1
    assert n_found == len(patches), (n_found, len(patches))
    return nc


# ---------------------------------------------------------------------------
_CACHED = {}


def _get_nc(cfg_key):
    if cfg_key not in _CACHED:
        _CACHED[cfg_key] = build_nc(**dict(cfg_key))
    return _CACHED[cfg_key]


def kernel(input_x, W, bias, coeffs):
    cfg = dict(FULL)
    n_cores, B = cfg["n_cores"], cfg["B"]
    IC, CH, OC = cfg["IC"], cfg["CH"], cfg["OC"]
    assert input_x.shape == (n_cores * B, IC, CH)

    nc = _get_nc(tuple(sorted(cfg.items())))

    w_f = np.asarray(W, dtype=np.float32)
    w_bf = w_f.astype(ml_dtypes.bfloat16)
    wt8 = np.ascontiguousarray(w_f.T).astype(ml_dtypes.float8_e4m3fn)
    bias_f = np.ascontiguousarray(np.asarray(bias, dtype=np.float32))
    coef_f = np.ascontiguousarray(
        np.asarray(coeffs, dtype=np.float32).reshape(IC))
    x = np.asarray(input_x, dtype=np.float32)

    in_maps = []
    for r in range(n_cores):
        xs = x[r * B:(r + 1) * B]                     # [B, IC, CH]
        xT = np.ascontiguousarray(xs.transpose(2, 0, 1)).reshape(CH, B * IC)
        xI = np.ascontiguousarray(xs.transpose(1, 0, 2)).reshape(IC, B * CH)
        xI8 = xI.astype(ml_dtypes.float8_e4m3fn)
        xR8 = (16.0 * (xI - xI8.astype(np.float32))).astype(
            ml_dtypes.float8_e4m3fn)
        xbar = (xs.astype(np.float64).sum(axis=1).T / IC)  # [CH, B]
        in_maps.append({
            "xT8": xT.astype(ml_dtypes.float8_e4m3fn),
            "xI8": xI8,
            "xR8": xR8,
            "Wt": w_bf,
            "WT8": wt8,
            "bias": bias_f,
            "coeffs": coef_f,
            "xbar": np.ascontiguousarray(xbar).astype(ml_dtypes.bfloat16),
        })

    try:  # NTFF tracing needs antenv.axon_hooks; drop BASS_TRACE if absent
        from antenv import axon_hooks  # noqa: F401
    except ImportError:
        os.environ.pop("BASS_TRACE", None)
    res = run_bass_kernel_spmd(nc, in_maps, core_ids=list(range(n_cores)))
    kernel.last_results = res
    out = np.concatenate([res.results[r]["vj_out"] for r in range(n_cores)],
                         axis=0)
    return out.astype(np.float32)


kernel.last_results = None
